# revision 1
# baseline (speedup 1.0000x reference)
"""Trainium2 Bass kernel for nn_DocREModel (doc-level relation extraction graph pooling).

Strategy (8 NeuronCores): each doc b (B=4) is split across 2 cores by attention
heads (6 heads each).  Every use of the attention tensor in the model is linear
in attention up to cheap scalar normalizations, so each core:
  - streams its [6,1024,1024] attention slice once from HBM,
  - accumulates the head-sum S[L,L] in SBUF (first head DMA'd straight into the
    accumulator, remaining heads added on the vector engine),
  - computes, via PE matmuls against host-built gather/mask matrices:
      GT     = S^T @ [onehotT|maskT]  (mention rows of S + span-row sums, both
                                       in contraction-major layout)
      v      = (uT*maskT)^T @ [seq|1]          (link-span numerator)
      mnum   = mrowsT^T @ [seq|1]              (mention-context numerator + row-sum)
      memb   = onehotT^T @ [seq|1]             (mention embeddings)
The host adds the two per-doc partials and applies the tiny normalizations
(head-count / span-length / row-sum divides, entity pooling, 4-way logsumexp)
while unsharding.
"""

import os
import sys

for _p in ("/opt/trn_rl_repo", "/root/.axon_site/_ro/trn_rl_repo"):
    if os.path.isdir(_p) and _p not in sys.path:
        sys.path.insert(0, _p)

import numpy as np

B, L, H, NH = 4, 1024, 768, 12
E, MPE, K = 32, 4, 16
EM = E * MPE              # 128 mentions per doc
TYPE_DIM = 20
OFFSET = 1
HPG = NH // 2             # heads per core (2 cores per doc)
RC = L // 128              # 8 chunks of 128 along L
HA = H + 2                # hidden + ones column (row-sum) + zero pad (fp32r needs even N)
RW = EM + K               # 144 real columns of the combined gather/mask matrix
RWP = 256                 # padded width so fp32r GT matmuls hit the 1cyc/row path

F32R_BIG = True           # float32r for the N>=256 contraction matmuls
F32R_GT = True            # float32r for the GT (S^T @ rmat) matmuls


def _build_nc(debug=False, f32r_big=F32R_BIG, f32r_gt=F32R_GT):
    import concourse.bass as bass
    import concourse.mybir as mybir
    import concourse.tile as tile
    from concourse import bacc

    f32 = mybir.dt.float32
    f32r = mybir.dt.float32r
    bf16 = mybir.dt.bfloat16
    ts, ds = bass.ts, bass.ds

    dm = f32r if (f32r_big or f32r_gt) else f32   # dtype for matmul operands

    def big(ap):
        return ap

    def gtc(ap):
        return ap

    nc = bacc.Bacc("TRN2", target_bir_lowering=False, debug=debug)

    att6 = nc.dram_tensor("att6", [HPG * L, L], bf16, kind="ExternalInput")
    seq_aug = nc.dram_tensor("seq_aug", [L, HA], dm, kind="ExternalInput")
    rmat = nc.dram_tensor("rmat", [L, RWP], dm, kind="ExternalInput")
    out_v = nc.dram_tensor("out_v", [K, HA], f32, kind="ExternalOutput")
    out_mnum = nc.dram_tensor("out_mnum", [EM, HA], f32, kind="ExternalOutput")
    out_memb = nc.dram_tensor("out_memb", [EM, HA], f32, kind="ExternalOutput")

    with tile.TileContext(nc) as tc:
        with (
            tc.tile_pool(name="const", bufs=1) as constp,
            tc.tile_pool(name="stream", bufs=12) as streamp,
            tc.tile_pool(name="accum", bufs=1) as accp,
            tc.tile_pool(name="stage", bufs=1) as stagep,
            tc.tile_pool(name="psall", bufs=8, space="PSUM") as psall,
        ):
            # ---- attention stream starts first (h=0 straight into S); consts
            #      interleave behind it so the HBM stream isn't delayed ----
            S_tiles = [accp.tile([128, L], dm, tag=f"S{rc}", name=f"S{rc}") for rc in range(RC)]
            gt_s = [accp.tile([128, RW], dm, tag=f"gt{ct}", name=f"gt{ct}") for ct in range(RC)]

            # consts loaded on the scalar queue right behind the first stream quad
            seq_s = constp.tile([128, RC, HA], dm, tag="seqs", name="seqs")
            rmat_s = constp.tile([128, RC, RWP], dm, tag="rmats", name="rmats")

            att6_r2 = att6[:].rearrange("(h rcq p) c -> h rcq p c", h=HPG, p=128)
            q0_tiles = []
            for h in range(HPG):
                t = streamp.tile([128, 4, L], bf16, tag="att", name="att")
                nc.sync.dma_start(out=t[:], in_=att6_r2[h, ds(0, 4)].rearrange("rcq p c -> p rcq c"))
                q0_tiles.append(t)
            for rc in range(RC):
                nc.scalar.dma_start(out=seq_s[:, rc, :], in_=seq_aug[ts(rc, 128), :])
                nc.scalar.dma_start(out=rmat_s[:, rc, :], in_=rmat[ts(rc, 128), :])

            # ---- mention embeddings memb = onehot^T @ [seq|1] (needs only consts) ----
            pmemb0 = psall.tile([EM, 512], f32, tag="ps", name="pmemb0")
            pmemb1 = psall.tile([EM, HA - 512], f32, tag="ps", name="pmemb1")
            for rc in range(RC):
                nc.tensor.matmul(pmemb0[:], big(rmat_s[:, rc, 0:EM]), big(seq_s[:, rc, 0:512]),
                                 start=(rc == 0), stop=(rc == RC - 1))
                nc.tensor.matmul(pmemb1[:], big(rmat_s[:, rc, 0:EM]), big(seq_s[:, rc, 512:HA]),
                                 start=(rc == 0), stop=(rc == RC - 1))
            memb_s = stagep.tile([EM, HA], f32, tag="memb", name="memb")
            nc.scalar.copy(out=memb_s[:, 0:512], in_=pmemb0[:])
            nc.scalar.copy(out=memb_s[:, 512:HA], in_=pmemb1[:])
            nc.sync.dma_start(out=out_memb[:], in_=memb_s[:])

            att6_r = att6[:].rearrange("(h rcq p) c -> h rcq p c", h=HPG, p=128)
            NQ = RC // 4  # two quads of four row-chunks
            groups = [list(range(0, 7)), list(range(7, 8))]  # GT groups: 7 + 1 chunks
            done_upto = 0
            for qq in range(NQ):
                if qq == 0:
                    tiles = q0_tiles
                else:
                    tiles = []
                    for h in range(HPG):
                        t = streamp.tile([128, 4, L], bf16, tag="att", name="att")
                        nc.sync.dma_start(out=t[:], in_=att6_r[h, ds(4 * qq, 4)].rearrange("rcq p c -> p rcq c"))
                        tiles.append(t)
                for j in range(4):
                    rc = 4 * qq + j
                    tp01 = streamp.tile([128, L], bf16, tag="tp", name="tp01", bufs=6)
                    tp23 = streamp.tile([128, L], bf16, tag="tp", name="tp23", bufs=6)
                    tp45 = streamp.tile([128, L], bf16, tag="tp", name="tp45", bufs=6)
                    nc.vector.tensor_add(tp01[:], tiles[0][:, j, :], tiles[1][:, j, :])
                    nc.vector.tensor_add(tp23[:], tiles[2][:, j, :], tiles[3][:, j, :])
                    nc.vector.tensor_add(tp45[:], tiles[4][:, j, :], tiles[5][:, j, :])
                    nc.vector.tensor_add(S_tiles[rc][:], tp01[:], tp23[:])
                    nc.vector.tensor_add(S_tiles[rc][:], S_tiles[rc][:], tp45[:])
                # GT group matmuls for every group fully covered by streamed chunks
                avail = 4 * qq + 4
                for gi, grp in enumerate(groups):
                    if grp[-1] < done_upto or grp[-1] >= avail:
                        continue
                    for ct in range(RC):
                        p = psall.tile([128, RWP], f32, tag="ps", name="gtq")
                        for j, rc in enumerate(grp):
                            nc.tensor.matmul(p[:], gtc(S_tiles[rc][:, ts(ct, 128)]), gtc(rmat_s[:, rc, :]),
                                             start=(j == 0), stop=(j == len(grp) - 1))
                        if gi == 0:
                            nc.scalar.copy(out=gt_s[ct][:], in_=p[:, 0:RW])
                        else:
                            nc.vector.tensor_add(gt_s[ct][:], gt_s[ct][:], p[:, 0:RW])
                    done_upto = grp[-1] + 1

            # ---- wvT = uT * maskT ----
            wv_s = [accp.tile([128, K], dm, tag=f"wv{ct}", name=f"wv{ct}") for ct in range(RC)]
            for ct in range(RC):
                nc.vector.tensor_mul(wv_s[ct][:], gt_s[ct][:, EM:RW], rmat_s[:, ct, EM:RW])

            # ---- contraction over positions: numerators for contexts + link reps ----
            pmnum0 = psall.tile([EM, 512], f32, tag="ps", name="pmnum0")
            pmnum1 = psall.tile([EM, HA - 512], f32, tag="ps", name="pmnum1")
            pv0 = psall.tile([K, 512], f32, tag="ps", name="pv0")
            pv1 = psall.tile([K, HA - 512], f32, tag="ps", name="pv1")
            for ct in range(RC):
                nc.tensor.matmul(pmnum0[:], big(gt_s[ct][:, 0:EM]), big(seq_s[:, ct, 0:512]),
                                 start=(ct == 0), stop=(ct == RC - 1))
                nc.tensor.matmul(pmnum1[:], big(gt_s[ct][:, 0:EM]), big(seq_s[:, ct, 512:HA]),
                                 start=(ct == 0), stop=(ct == RC - 1))
                nc.tensor.matmul(pv0[:], big(wv_s[ct][:]), big(seq_s[:, ct, 0:512]),
                                 start=(ct == 0), stop=(ct == RC - 1))
                nc.tensor.matmul(pv1[:], big(wv_s[ct][:]), big(seq_s[:, ct, 512:HA]),
                                 start=(ct == 0), stop=(ct == RC - 1))
            mnum_s = stagep.tile([EM, HA], f32, tag="mnum", name="mnum")
            nc.scalar.copy(out=mnum_s[:, 0:512], in_=pmnum0[:])
            nc.scalar.copy(out=mnum_s[:, 512:HA], in_=pmnum1[:])
            nc.sync.dma_start(out=out_mnum[:], in_=mnum_s[:])
            v_s = stagep.tile([K, HA], f32, tag="v", name="v")
            nc.scalar.copy(out=v_s[:, 0:512], in_=pv0[:])
            nc.scalar.copy(out=v_s[:, 512:HA], in_=pv1[:])
            nc.scalar.dma_start(out=out_v[:], in_=v_s[:])

    nc.compile()
    return nc


_NC_CACHE = {}


def _get_nc():
    if "nc" not in _NC_CACHE:
        _NC_CACHE["nc"] = _build_nc()
    return _NC_CACHE["nc"]


def _per_core_inputs(sequence_output, attention, mention_pos, link_start, link_len):
    """Returns (in_maps for 8 cores, per-doc span lengths)."""
    seq = np.ascontiguousarray(np.asarray(sequence_output, dtype=np.float32))
    import ml_dtypes
    att = np.asarray(attention)
    mpos = np.asarray(mention_pos).astype(np.int64)
    lstart = np.asarray(link_start).astype(np.int64)
    llen = np.asarray(link_len).astype(np.int64)

    in_maps = []
    lengths = []
    for b in range(B):
        pos = (mpos[b] + OFFSET).reshape(EM)
        onehotT = np.zeros((L, EM), np.float32)
        onehotT[pos, np.arange(EM)] = 1.0
        s = lstart[b] + OFFSET
        e = lstart[b] + llen[b] + 1 + OFFSET
        r = np.arange(L)
        maskT = ((r[:, None] >= s[None, :]) & (r[:, None] < e[None, :])).astype(np.float32)
        rmat = np.ascontiguousarray(np.concatenate(
            [onehotT, maskT, np.zeros((L, RWP - RW), np.float32)], axis=1))
        seq_aug = np.ascontiguousarray(
            np.concatenate([seq[b], np.ones((L, 1), np.float32), np.zeros((L, 1), np.float32)], axis=1))
        lengths.append((e - s).astype(np.float32))
        for g in range(2):
            att6 = np.ascontiguousarray(
                att[b, g * HPG:(g + 1) * HPG].reshape(HPG * L, L).astype(ml_dtypes.bfloat16))
            in_maps.append({"att6": att6, "seq_aug": seq_aug, "rmat": rmat})
    return in_maps, lengths


def _combine(outs, lengths, type_table):
    ttab = np.asarray(type_table, dtype=np.float32)
    type_ids = np.concatenate(
        [np.zeros(E, np.int64), np.ones(EM, np.int64), np.full(K, 2, np.int64)])
    nodes_type = ttab[type_ids]  # [E+EM+K, TYPE_DIM]

    out = np.zeros((B, E + EM + K + E + EM, H + TYPE_DIM), np.float32)
    for b in range(B):
        o0, o1 = outs[2 * b], outs[2 * b + 1]
        v = o0["out_v"] + o1["out_v"]
        mnum = o0["out_mnum"] + o1["out_mnum"]
        memb = o0["out_memb"][:, :H]
        length = lengths[b]

        link_rep = v[:, :H] / (NH * length[:, None])
        m_ctx = mnum[:, :H] / (mnum[:, H:H + 1] + NH * 1e-5)
        enum = mnum.reshape(E, MPE, HA).sum(axis=1)
        e_ctx = enum[:, :H] / (enum[:, H:H + 1] + NH * MPE * 1e-5)

        mg = memb.reshape(E, MPE, H)
        mmax = mg.max(axis=1)
        eemb = np.log(np.exp(mg - mmax[:, None, :]).sum(axis=1)) + mmax

        nodes_raw = np.concatenate([eemb, memb, link_rep], axis=0)      # [176,H]
        nodes = np.concatenate([nodes_raw, nodes_type], axis=1)         # [176,H+20]
        ctx = np.concatenate([e_ctx, m_ctx], axis=0)                    # [160,H]
        ctx = np.concatenate([ctx, np.zeros((E + EM, TYPE_DIM), np.float32)], axis=1)
        out[b] = np.concatenate([nodes, ctx], axis=0)
    return out


def kernel(**inputs):
    from concourse.bass_utils import run_bass_kernel_spmd

    in_maps, lengths = _per_core_inputs(
        inputs["sequence_output"], inputs["attention"],
        inputs["mention_pos"], inputs["link_start"], inputs["link_len"])
    nc = _get_nc()
    res = run_bass_kernel_spmd(nc, in_maps, core_ids=list(range(8)))
    return _combine(res.results, lengths, inputs["type_table"])



# revision 10
# speedup vs baseline: 1.5973x; 1.5973x over previous
"""Trainium2 Bass kernel for nn_DocREModel (doc-level relation extraction graph pooling).

Strategy (8 NeuronCores): each doc b (B=4) is split across 2 cores by attention
heads (6 heads each).  Key observation: the model only ever reads attention rows
at mention positions (<=128 distinct) and inside link spans (<=496), i.e. ~35%
of the [1024,1024] matrix.  Each core therefore device-GATHERS just those rows
(SWDGE dma_gather with a runtime index tensor), head-sums them on the vector
engine, and runs small PE matmuls:

  - att_r HBM layout [row, head*1024+col] (bf16): gathering one row pulls all 6
    heads of that row contiguously (12 KB/descriptor).
  - gather slot j<128 == mention j (EM=128 exactly), so the "onehot" gather
    matrix is the identity: mention rows of S come from a PE transpose of
    gathered chunk 0.  Span rows occupy slots >=128; a host-built slot mask
    [slots,16] reduces them to per-span row-sums u_k via PE matmul.
  - mnum = S_mention^T @ [seq|1]  (context numerators + row-sums)
  - v    = (u*colmask)^T @ [seq|1] (link span numerators)
  - memb = seq rows at mentions via a second (f32) dma_gather - no compute.
The host applies the tiny normalizations (head-count / span-length / row-sum
divides, entity pooling, 4-way logsumexp) while unsharding.
"""

import os
import sys

for _p in ("/opt/trn_rl_repo", "/root/.axon_site/_ro/trn_rl_repo"):
    if os.path.isdir(_p) and _p not in sys.path:
        sys.path.insert(0, _p)

import numpy as np

B, L, H, NH = 4, 1024, 768, 12
E, MPE, K = 32, 4, 16
EM = E * MPE              # 128 mentions per doc == gather chunk 0
TYPE_DIM = 20
OFFSET = 1
HPG = NH // 2             # heads per core (2 cores per doc)
CT = L // 128             # 8 column chunks
HA = H + 4                # seq | ones | 3 zero-pad -> 772 (row-sum in col 768)
NCHUNK_DEF = 3            # gather slots = 128*NCHUNK (>= 128 mentions + span rows)


def _build_nc(nchunk=NCHUNK_DEF, debug=False):
    import concourse.bass as bass
    import concourse.mybir as mybir
    import concourse.tile as tile
    from concourse import bacc

    f32 = mybir.dt.float32
    bf16 = mybir.dt.bfloat16
    i16 = mybir.dt.int16
    ts, ds = bass.ts, bass.ds

    nc = bacc.Bacc("TRN2", target_bir_lowering=False, debug=debug)

    att_r = nc.dram_tensor("att_r", [L, HPG * L], bf16, kind="ExternalInput")
    seqb = nc.dram_tensor("seqb", [L, HA], bf16, kind="ExternalInput")
    seqf = nc.dram_tensor("seqf", [L, H], f32, kind="ExternalInput")
    idx_att = nc.dram_tensor("idx_att", [128, nchunk * 8], i16, kind="ExternalInput")
    idx_mem = nc.dram_tensor("idx_mem", [128, 8], i16, kind="ExternalInput")
    maskS = nc.dram_tensor("maskS", [128, nchunk * K], bf16, kind="ExternalInput")
    maskCT = nc.dram_tensor("maskCT", [K, L], f32, kind="ExternalInput")
    ident = nc.dram_tensor("ident", [128, 128], bf16, kind="ExternalInput")
    out_mnum = nc.dram_tensor("out_mnum", [EM, HA], f32, kind="ExternalOutput")
    out_v = nc.dram_tensor("out_v", [K, HA], f32, kind="ExternalOutput")
    out_memb = nc.dram_tensor("out_memb", [EM, H], f32, kind="ExternalOutput")

    with tile.TileContext(nc) as tc:
        with (
            tc.tile_pool(name="const", bufs=1) as constp,
            tc.tile_pool(name="gat", bufs=2) as gatp,
            tc.tile_pool(name="tree", bufs=4) as treep,
            tc.tile_pool(name="acc", bufs=1) as accp,
            tc.tile_pool(name="stage", bufs=1) as stagep,
            tc.tile_pool(name="pshold", bufs=1, space="PSUM") as pshold,
            tc.tile_pool(name="psrot", bufs=2, space="PSUM") as psrot,
        ):
            # ---- consts + indices (small, front-loaded) ----
            idxa_s = constp.tile([128, nchunk * 8], i16, tag="idxa", name="idxa")
            idxm_s = constp.tile([128, 8], i16, tag="idxm", name="idxm")
            maskS_s = constp.tile([128, nchunk, K], bf16, tag="maskS", name="maskS")
            maskCT_s = constp.tile([K, L], f32, tag="maskCT", name="maskCT")
            ident_s = constp.tile([128, 128], bf16, tag="ident", name="ident")
            seq_s = constp.tile([128, CT, HA], bf16, tag="seqs", name="seqs")
            nc.sync.dma_start(out=idxa_s[:], in_=idx_att[:])
            nc.sync.dma_start(out=idxm_s[:], in_=idx_mem[:])
            nc.sync.dma_start(out=maskS_s[:], in_=maskS[:].rearrange("p (q k) -> p q k", k=K))
            nc.scalar.dma_start(out=maskCT_s[:], in_=maskCT[:])
            nc.sync.dma_start(out=ident_s[:], in_=ident[:])
            nc.scalar.dma_start(out=seq_s[:], in_=seqb[:].rearrange("(c p) f -> p c f", p=128))

            # ---- mention embeddings: pure f32 row-gather of seq ----
            memb_s = stagep.tile([128, 1, H], f32, tag="memb", name="memb")
            nc.gpsimd.dma_gather(memb_s[:], seqf[:], idxm_s[:], 128, 128, H)
            nc.sync.dma_start(out=out_memb[:], in_=memb_s[:, 0, :])

            # ---- gather attention rows chunk-by-chunk; head-sum tree on DVE ----
            gs_s = accp.tile([128, nchunk, L], bf16, tag="gs", name="gs")
            gtm_s = accp.tile([128, CT, 128], bf16, tag="gtm", name="gtm")
            wv_s = accp.tile([128, CT, K], bf16, tag="wv", name="wv")
            # uT[k,c] accumulators: one PSUM accumulation group per bank
            put0 = pshold.tile([K, 512], f32, tag="put0", name="put0")
            put1 = pshold.tile([K, 512], f32, tag="put1", name="put1")
            for q in range(nchunk):
                g5 = gatp.tile([128, 1, HPG * L], bf16, tag="g5", name="g5")
                nc.gpsimd.dma_gather(g5[:], att_r[:], idxa_s[:, ds(q * 8, 8)],
                                     128, 128, HPG * L)
                ta = treep.tile([128, L], bf16, tag="tp", name="ta")
                tb = treep.tile([128, L], bf16, tag="tp", name="tb")
                tc_ = treep.tile([128, L], bf16, tag="tp", name="tc")
                td = treep.tile([128, L], bf16, tag="tp", name="td")
                nc.vector.tensor_add(ta[:], g5[:, 0, ds(0, L)], g5[:, 0, ds(L, L)])
                nc.vector.tensor_add(tb[:], g5[:, 0, ds(2 * L, L)], g5[:, 0, ds(3 * L, L)])
                nc.vector.tensor_add(tc_[:], g5[:, 0, ds(4 * L, L)], g5[:, 0, ds(5 * L, L)])
                nc.vector.tensor_add(td[:], ta[:], tb[:])
                nc.vector.tensor_add(gs_s[:, q, :], td[:], tc_[:])
                # span-row reduction (transposed): uT[k,c] += maskS^T @ gs_q
                nc.tensor.matmul(put0[:], maskS_s[:, q, :], gs_s[:, q, 0:512],
                                 start=(q == 0), stop=(q == nchunk - 1))
                nc.tensor.matmul(put1[:], maskS_s[:, q, :], gs_s[:, q, 512:L],
                                 start=(q == 0), stop=(q == nchunk - 1))
                if q == 0:
                    # mention rows of S: PE-transpose of gathered chunk 0
                    for ct in range(CT):
                        pt = psrot.tile([128, 128], bf16, tag="pt", name="pt")
                        nc.tensor.transpose(pt[:], gs_s[:, 0, ts(ct, 128)], ident_s[:])
                        nc.scalar.copy(out=gtm_s[:, ct, :], in_=pt[:])

            # ---- wvT = uT * column-mask, then transpose back to [c,k] chunks ----
            wvt_s = accp.tile([K, L], bf16, tag="wvt", name="wvt")
            nc.vector.tensor_mul(wvt_s[:, 0:512], put0[:], maskCT_s[:, 0:512])
            nc.vector.tensor_mul(wvt_s[:, 512:L], put1[:], maskCT_s[:, 512:L])
            for ct in range(CT):
                ptk = psrot.tile([128, 128], bf16, tag="pt", name="ptk")
                nc.tensor.transpose(ptk[:, 0:K], wvt_s[:, ts(ct, 128)], ident_s[0:K, 0:K])
                nc.scalar.copy(out=wv_s[:, ct, :], in_=ptk[:, 0:K])

            # ---- contraction over positions: mnum (mention ctx) + v (links) ----
            pm0 = pshold.tile([EM, 512], f32, tag="pm0", name="pm0")
            pm1 = pshold.tile([EM, HA - 512], f32, tag="pm1", name="pm1")
            pv0 = pshold.tile([K, 512], f32, tag="pv0", name="pv0")
            pv1 = pshold.tile([K, HA - 512], f32, tag="pv1", name="pv1")
            for ct in range(CT):
                st, sp = (ct == 0), (ct == CT - 1)
                nc.tensor.matmul(pm0[:], gtm_s[:, ct, :], seq_s[:, ct, 0:512], start=st, stop=sp)
                nc.tensor.matmul(pm1[:], gtm_s[:, ct, :], seq_s[:, ct, 512:HA], start=st, stop=sp)
                nc.tensor.matmul(pv0[:], wv_s[:, ct, :], seq_s[:, ct, 0:512], start=st, stop=sp)
                nc.tensor.matmul(pv1[:], wv_s[:, ct, :], seq_s[:, ct, 512:HA], start=st, stop=sp)
            mnum_s = stagep.tile([EM, HA], f32, tag="mnum", name="mnum")
            nc.scalar.copy(out=mnum_s[:, 0:512], in_=pm0[:])
            nc.scalar.copy(out=mnum_s[:, 512:HA], in_=pm1[:])
            nc.sync.dma_start(out=out_mnum[:], in_=mnum_s[:])
            v_s = stagep.tile([K, HA], f32, tag="v", name="v")
            nc.scalar.copy(out=v_s[:, 0:512], in_=pv0[:])
            nc.scalar.copy(out=v_s[:, 512:HA], in_=pv1[:])
            nc.scalar.dma_start(out=out_v[:], in_=v_s[:])

    nc.compile()
    return nc


_NC_CACHE = {}


def _get_nc(nchunk=NCHUNK_DEF):
    if nchunk not in _NC_CACHE:
        _NC_CACHE[nchunk] = _build_nc(nchunk)
    return _NC_CACHE[nchunk]


def _wrap_idx(idx, width):
    """int16 index list -> [128, width] wrapped (k at [k%16, k//16], x8 replicated)."""
    a = np.zeros(16 * width, np.int16)
    a[: len(idx)] = idx
    return np.tile(a.reshape(width, 16).T, (8, 1)).copy()


def _per_core_inputs(sequence_output, attention, mention_pos, link_start, link_len):
    """Returns (in_maps for 8 cores, per-doc span lengths, nchunk)."""
    import ml_dtypes
    seq = np.ascontiguousarray(np.asarray(sequence_output, dtype=np.float32))
    att = np.asarray(attention)
    mpos = np.asarray(mention_pos).astype(np.int64)
    lstart = np.asarray(link_start).astype(np.int64)
    llen = np.asarray(link_len).astype(np.int64)

    docs = []
    max_slots = 0
    for b in range(B):
        pos = (mpos[b] + OFFSET).reshape(EM)
        s = lstart[b] + OFFSET
        e = lstart[b] + llen[b] + 1 + OFFSET
        row2slot = {}
        slots = list(pos)
        for j, r in enumerate(pos):
            row2slot.setdefault(int(r), j)
        for si, ei in zip(s, e):
            for r in range(int(si), int(ei)):
                if r not in row2slot:
                    row2slot[r] = len(slots)
                    slots.append(r)
        docs.append((pos, s, e, row2slot, slots))
        max_slots = max(max_slots, len(slots))
    nchunk = max(NCHUNK_DEF, -(-max_slots // 128))

    ident = np.eye(128, dtype=ml_dtypes.bfloat16)
    in_maps = []
    lengths = []
    for b in range(B):
        pos, s, e, row2slot, slots = docs[b]
        n_slots = nchunk * 128
        idx_att = _wrap_idx(np.asarray(slots, np.int16), nchunk * 8)
        idx_mem = _wrap_idx(pos.astype(np.int16), 8)
        mS = np.zeros((n_slots, K), np.float32)
        mC = np.zeros((L, K), np.float32)
        for k, (si, ei) in enumerate(zip(s, e)):
            mC[int(si):int(ei), k] = 1.0
            for r in range(int(si), int(ei)):
                mS[row2slot[r], k] = 1.0
        maskS = np.ascontiguousarray(
            mS.reshape(nchunk, 128, K).transpose(1, 0, 2).reshape(128, nchunk * K)
        ).astype(ml_dtypes.bfloat16)
        maskCT = np.ascontiguousarray(mC.T)
        seqb = np.concatenate(
            [seq[b], np.ones((L, 1), np.float32), np.zeros((L, HA - H - 1), np.float32)],
            axis=1).astype(ml_dtypes.bfloat16)
        seqf = seq[b]
        lengths.append((e - s).astype(np.float32))
        for g in range(2):
            att_r = np.ascontiguousarray(
                att[b, g * HPG:(g + 1) * HPG].astype(ml_dtypes.bfloat16)
                .transpose(1, 0, 2).reshape(L, HPG * L))
            in_maps.append({
                "att_r": att_r, "seqb": seqb, "seqf": seqf,
                "idx_att": idx_att, "idx_mem": idx_mem,
                "maskS": maskS, "maskCT": maskCT, "ident": ident,
            })
    return in_maps, lengths, nchunk


def _combine(outs, lengths, type_table):
    ttab = np.asarray(type_table, dtype=np.float32)
    type_ids = np.concatenate(
        [np.zeros(E, np.int64), np.ones(EM, np.int64), np.full(K, 2, np.int64)])
    nodes_type = ttab[type_ids]  # [E+EM+K, TYPE_DIM]

    out = np.zeros((B, E + EM + K + E + EM, H + TYPE_DIM), np.float32)
    for b in range(B):
        o0, o1 = outs[2 * b], outs[2 * b + 1]
        v = o0["out_v"] + o1["out_v"]
        mnum = o0["out_mnum"] + o1["out_mnum"]
        memb = o0["out_memb"]
        length = lengths[b]

        link_rep = v[:, :H] / (NH * length[:, None])
        m_ctx = mnum[:, :H] / (mnum[:, H:H + 1] + NH * 1e-5)
        enum = mnum.reshape(E, MPE, HA).sum(axis=1)
        e_ctx = enum[:, :H] / (enum[:, H:H + 1] + NH * MPE * 1e-5)

        mg = memb.reshape(E, MPE, H)
        mmax = mg.max(axis=1)
        eemb = np.log(np.exp(mg - mmax[:, None, :]).sum(axis=1)) + mmax

        nodes_raw = np.concatenate([eemb, memb, link_rep], axis=0)      # [176,H]
        nodes = np.concatenate([nodes_raw, nodes_type], axis=1)         # [176,H+20]
        ctx = np.concatenate([e_ctx, m_ctx], axis=0)                    # [160,H]
        ctx = np.concatenate([ctx, np.zeros((E + EM, TYPE_DIM), np.float32)], axis=1)
        out[b] = np.concatenate([nodes, ctx], axis=0)
    return out


def kernel(**inputs):
    from concourse.bass_utils import run_bass_kernel_spmd

    in_maps, lengths, nchunk = _per_core_inputs(
        inputs["sequence_output"], inputs["attention"],
        inputs["mention_pos"], inputs["link_start"], inputs["link_len"])
    nc = _get_nc(nchunk)
    res = run_bass_kernel_spmd(nc, in_maps, core_ids=list(range(8)))
    return _combine(res.results, lengths, inputs["type_table"])


# revision 14
# speedup vs baseline: 1.7858x; 1.1180x over previous
"""Trainium2 Bass kernel for nn_DocREModel (doc-level relation extraction graph pooling).

Strategy (8 NeuronCores): each doc b (B=4) is split across 2 cores by attention
heads (6 heads each).  Key observation: the model only ever reads attention rows
at mention positions (<=128 distinct) and inside link spans, i.e. ~35% of the
[1024,1024] matrix.  Each core therefore device-GATHERS just those rows
(SWDGE indirect DMA with a runtime per-partition index tile), head-sums them on
the vector engine, and runs small PE matmuls:

  - att_r HBM layout [row, head*1024+col] (bf16): gathering one row pulls all 6
    heads of that row contiguously (12 KB/descriptor).
  - gather slot j<128 == mention j (EM=128 exactly), so the "onehot" gather
    matrix is the identity: mention rows of S come from a PE transpose of
    gathered chunk 0.  Span rows occupy slots >=128; a host-built slot mask
    [slots,16] reduces them to per-span row-sums uT[16,L] via PE matmul
    (transposed so each PSUM bank holds a single accumulation group).
  - mnum = S_mention^T @ [seq|1]  (context numerators + row-sums)
  - v    = (uT*maskCT)^T-transposed-back @ [seq|1], col-group-tiled 4x on the
    PE (16-row outputs packed at partition offsets 0/32/64/96; host sums).
  - memb = seq rows at mentions via a second (f32) indirect gather - no compute.
The host applies the tiny normalizations (head-count / span-length / row-sum
divides, entity pooling, 4-way logsumexp) while unsharding.
"""

import os
import sys

for _p in ("/opt/trn_rl_repo", "/root/.axon_site/_ro/trn_rl_repo"):
    if os.path.isdir(_p) and _p not in sys.path:
        sys.path.insert(0, _p)

import numpy as np

B, L, H, NH = 4, 1024, 768, 12
E, MPE, K = 32, 4, 16
EM = E * MPE              # 128 mentions per doc == gather chunk 0
TYPE_DIM = 20
OFFSET = 1
HPG = NH // 2             # heads per core (2 cores per doc)
CT = L // 128             # 8 column chunks
HA = H + 4                # seq | ones | 3 zero-pad -> 772 (row-sum in col 768)
NCHUNK_DEF = 3            # gather slots = 128*NCHUNK (>= 128 mentions + span rows)


def _build_nc(nchunk=NCHUNK_DEF, debug=False):
    import concourse.bass as bass
    import concourse.mybir as mybir
    import concourse.tile as tile
    from concourse import bacc

    f32 = mybir.dt.float32
    bf16 = mybir.dt.bfloat16
    i32 = mybir.dt.int32
    ts, ds = bass.ts, bass.ds

    nc = bacc.Bacc("TRN2", target_bir_lowering=False, debug=debug)

    att_r = nc.dram_tensor("att_r", [L, HPG * L], bf16, kind="ExternalInput")
    seqb = nc.dram_tensor("seqb", [L, HA], bf16, kind="ExternalInput")
    seqf = nc.dram_tensor("seqf", [L, H], f32, kind="ExternalInput")
    idx_att = nc.dram_tensor("idx_att", [128, nchunk], i32, kind="ExternalInput")
    idx_mem = nc.dram_tensor("idx_mem", [128, 1], i32, kind="ExternalInput")
    maskS = nc.dram_tensor("maskS", [128, nchunk * K], bf16, kind="ExternalInput")
    maskCT = nc.dram_tensor("maskCT", [K, L], f32, kind="ExternalInput")
    ident = nc.dram_tensor("ident", [128, 128], bf16, kind="ExternalInput")
    out_mnum = nc.dram_tensor("out_mnum", [EM, HA], f32, kind="ExternalOutput")
    out_v = nc.dram_tensor("out_v", [128, HA], f32, kind="ExternalOutput")
    out_memb = nc.dram_tensor("out_memb", [EM, H], f32, kind="ExternalOutput")

    with tile.TileContext(nc) as tc:
        with (
            tc.tile_pool(name="const", bufs=1) as constp,
            tc.tile_pool(name="gat", bufs=2) as gatp,
            tc.tile_pool(name="tree", bufs=4) as treep,
            tc.tile_pool(name="acc", bufs=1) as accp,
            tc.tile_pool(name="stage", bufs=1) as stagep,
            tc.tile_pool(name="pshold", bufs=1, space="PSUM") as pshold,
            tc.tile_pool(name="psrot", bufs=2, space="PSUM") as psrot,
        ):
            # ---- indices first: gathers are the critical path ----
            idxa_s = constp.tile([128, nchunk], i32, tag="idxa", name="idxa")
            idxm_s = constp.tile([128, 1], i32, tag="idxm", name="idxm")
            nc.sync.dma_start(out=idxa_s[:], in_=idx_att[:])
            nc.sync.dma_start(out=idxm_s[:], in_=idx_mem[:])

            gs_s = accp.tile([128, nchunk, L], bf16, tag="gs", name="gs")
            g5s = []
            for q in range(nchunk):
                g5 = gatp.tile([128, HPG * L], bf16, tag="g5", name="g5")
                nc.gpsimd.indirect_dma_start(
                    out=g5[:], out_offset=None, in_=att_r[:],
                    in_offset=bass.IndirectOffsetOnAxis(ap=idxa_s[:, q:q + 1], axis=0))
                g5s.append(g5)

            # ---- mention embeddings: pure f32 row-gather of seq ----
            memb_s = stagep.tile([128, H], f32, tag="memb", name="memb")
            nc.gpsimd.indirect_dma_start(
                out=memb_s[:], out_offset=None, in_=seqf[:],
                in_offset=bass.IndirectOffsetOnAxis(ap=idxm_s[:], axis=0))
            nc.sync.dma_start(out=out_memb[:], in_=memb_s[:])

            # ---- remaining consts ----
            maskS_s = constp.tile([128, nchunk, K], bf16, tag="maskS", name="maskS")
            maskCT_s = constp.tile([K, L], f32, tag="maskCT", name="maskCT")
            ident_s = constp.tile([128, 128], bf16, tag="ident", name="ident")
            seq_s = constp.tile([128, CT, HA], bf16, tag="seqs", name="seqs")
            nc.sync.dma_start(out=maskS_s[:], in_=maskS[:].rearrange("p (q k) -> p q k", k=K))
            nc.scalar.dma_start(out=maskCT_s[:], in_=maskCT[:])
            nc.sync.dma_start(out=ident_s[:], in_=ident[:])
            nc.scalar.dma_start(out=seq_s[:], in_=seqb[:].rearrange("(c p) f -> p c f", p=128))

            # ---- head-sum tree on DVE per chunk; span reduction + transposes on PE ----
            gtm_s = accp.tile([128, CT, 128], bf16, tag="gtm", name="gtm")
            wv_s = accp.tile([128, CT, K], bf16, tag="wv", name="wv")
            put0 = pshold.tile([K, 512], f32, tag="put0", name="put0")
            put1 = pshold.tile([K, 512], f32, tag="put1", name="put1")
            for q in range(nchunk):
                g5 = g5s[q]
                ta = treep.tile([128, L], bf16, tag="tp", name="ta")
                tb = treep.tile([128, L], bf16, tag="tp", name="tb")
                tc_ = treep.tile([128, L], bf16, tag="tp", name="tc")
                td = treep.tile([128, L], bf16, tag="tp", name="td")
                nc.vector.tensor_add(ta[:], g5[:, ds(0, L)], g5[:, ds(L, L)])
                nc.vector.tensor_add(tb[:], g5[:, ds(2 * L, L)], g5[:, ds(3 * L, L)])
                nc.vector.tensor_add(tc_[:], g5[:, ds(4 * L, L)], g5[:, ds(5 * L, L)])
                nc.vector.tensor_add(td[:], ta[:], tb[:])
                nc.vector.tensor_add(gs_s[:, q, :], td[:], tc_[:])
                # span-row reduction (transposed): uT[k,c] += maskS^T @ gs_q
                nc.tensor.matmul(put0[:], maskS_s[:, q, :], gs_s[:, q, 0:512],
                                 start=(q == 0), stop=(q == nchunk - 1))
                nc.tensor.matmul(put1[:], maskS_s[:, q, :], gs_s[:, q, 512:L],
                                 start=(q == 0), stop=(q == nchunk - 1))
                if q == 0:
                    # mention rows of S: PE-transpose of gathered chunk 0
                    for ct in range(CT):
                        pt = psrot.tile([128, 128], bf16, tag="pt", name="pt")
                        nc.tensor.transpose(pt[:], gs_s[:, 0, ts(ct, 128)], ident_s[:])
                        nc.scalar.copy(out=gtm_s[:, ct, :], in_=pt[:])

            # ---- mnum: mention-context numerators (runs early, off gtm) ----
            pm0 = pshold.tile([EM, 512], f32, tag="pm0", name="pm0")
            pm1 = pshold.tile([EM, HA - 512], f32, tag="pm1", name="pm1")
            for ct in range(CT):
                st, sp = (ct == 0), (ct == CT - 1)
                nc.tensor.matmul(pm0[:], gtm_s[:, ct, :], seq_s[:, ct, 0:512], start=st, stop=sp)
                nc.tensor.matmul(pm1[:], gtm_s[:, ct, :], seq_s[:, ct, 512:HA], start=st, stop=sp)
            mnum_s = stagep.tile([EM, HA], f32, tag="mnum", name="mnum")
            nc.scalar.copy(out=mnum_s[:, 0:512], in_=pm0[:])
            nc.scalar.copy(out=mnum_s[:, 512:HA], in_=pm1[:])
            nc.sync.dma_start(out=out_mnum[:], in_=mnum_s[:])

            # ---- wvT = uT * column-mask, transpose back, col-tiled v matmuls ----
            wvt_s = accp.tile([K, L], bf16, tag="wvt", name="wvt")
            nc.vector.tensor_mul(wvt_s[:, 0:512], put0[:], maskCT_s[:, 0:512])
            nc.vector.tensor_mul(wvt_s[:, 512:L], put1[:], maskCT_s[:, 512:L])
            for ct in range(CT):
                ptk = psrot.tile([128, 128], bf16, tag="pt", name="ptk")
                nc.tensor.transpose(ptk[:, 0:K], wvt_s[:, ts(ct, 128)], ident_s[0:K, 0:K])
                nc.vector.tensor_copy(wv_s[:, ct, :], ptk[:, 0:K])
            pv0 = pshold.tile([128, 512], f32, tag="pv0", name="pv0")
            pv1 = pshold.tile([128, HA - 512], f32, tag="pv1", name="pv1")
            for ct in range(CT):
                grp = ct // 2
                st, sp = (ct % 2 == 0), (ct % 2 == 1)
                nc.tensor.matmul(pv0[ds(32 * grp, K), :], wv_s[:, ct, :],
                                 seq_s[:, ct, 0:512], start=st, stop=sp,
                                 tile_position=(0, 32 * grp))
                nc.tensor.matmul(pv1[ds(32 * grp, K), :], wv_s[:, ct, :],
                                 seq_s[:, ct, 512:HA], start=st, stop=sp,
                                 tile_position=(0, 32 * grp))
            v_s = stagep.tile([128, HA], f32, tag="v", name="v")
            for grp in range(4):
                nc.scalar.copy(out=v_s[ds(32 * grp, K), 0:512], in_=pv0[ds(32 * grp, K), :])
                nc.scalar.copy(out=v_s[ds(32 * grp, K), 512:HA], in_=pv1[ds(32 * grp, K), :])
            nc.scalar.dma_start(out=out_v[:], in_=v_s[:])

    nc.compile()
    return nc


_NC_CACHE = {}


def _get_nc(nchunk=NCHUNK_DEF):
    if nchunk not in _NC_CACHE:
        _NC_CACHE[nchunk] = _build_nc(nchunk)
    return _NC_CACHE[nchunk]


def _per_core_inputs(sequence_output, attention, mention_pos, link_start, link_len):
    """Returns (in_maps for 8 cores, per-doc span lengths, nchunk)."""
    import ml_dtypes
    seq = np.ascontiguousarray(np.asarray(sequence_output, dtype=np.float32))
    att = np.asarray(attention)
    mpos = np.asarray(mention_pos).astype(np.int64)
    lstart = np.asarray(link_start).astype(np.int64)
    llen = np.asarray(link_len).astype(np.int64)

    docs = []
    max_slots = 0
    for b in range(B):
        pos = (mpos[b] + OFFSET).reshape(EM)
        s = lstart[b] + OFFSET
        e = lstart[b] + llen[b] + 1 + OFFSET
        row2slot = {}
        slots = list(pos)
        for j, r in enumerate(pos):
            row2slot.setdefault(int(r), j)
        for si, ei in zip(s, e):
            for r in range(int(si), int(ei)):
                if r not in row2slot:
                    row2slot[r] = len(slots)
                    slots.append(r)
        docs.append((pos, s, e, row2slot, slots))
        max_slots = max(max_slots, len(slots))
    nchunk = max(NCHUNK_DEF, -(-max_slots // 128))

    ident = np.eye(128, dtype=ml_dtypes.bfloat16)
    in_maps = []
    lengths = []
    for b in range(B):
        pos, s, e, row2slot, slots = docs[b]
        n_slots = nchunk * 128
        sl = np.zeros(n_slots, np.int32)
        sl[:len(slots)] = slots
        idx_att = np.ascontiguousarray(sl.reshape(nchunk, 128).T)
        idx_mem = np.ascontiguousarray(pos.astype(np.int32).reshape(128, 1))
        mS = np.zeros((n_slots, K), np.float32)
        mC = np.zeros((L, K), np.float32)
        for k, (si, ei) in enumerate(zip(s, e)):
            mC[int(si):int(ei), k] = 1.0
            for r in range(int(si), int(ei)):
                mS[row2slot[r], k] = 1.0
        maskS = np.ascontiguousarray(
            mS.reshape(nchunk, 128, K).transpose(1, 0, 2).reshape(128, nchunk * K)
        ).astype(ml_dtypes.bfloat16)
        maskCT = np.ascontiguousarray(mC.T)
        seqb = np.concatenate(
            [seq[b], np.ones((L, 1), np.float32), np.zeros((L, HA - H - 1), np.float32)],
            axis=1).astype(ml_dtypes.bfloat16)
        seqf = seq[b]
        lengths.append((e - s).astype(np.float32))
        for g in range(2):
            att_r = np.ascontiguousarray(
                att[b, g * HPG:(g + 1) * HPG].astype(ml_dtypes.bfloat16)
                .transpose(1, 0, 2).reshape(L, HPG * L))
            in_maps.append({
                "att_r": att_r, "seqb": seqb, "seqf": seqf,
                "idx_att": idx_att, "idx_mem": idx_mem,
                "maskS": maskS, "maskCT": maskCT, "ident": ident,
            })
    return in_maps, lengths, nchunk


def _combine(outs, lengths, type_table):
    ttab = np.asarray(type_table, dtype=np.float32)
    type_ids = np.concatenate(
        [np.zeros(E, np.int64), np.ones(EM, np.int64), np.full(K, 2, np.int64)])
    nodes_type = ttab[type_ids]  # [E+EM+K, TYPE_DIM]

    out = np.zeros((B, E + EM + K + E + EM, H + TYPE_DIM), np.float32)
    for b in range(B):
        o0, o1 = outs[2 * b], outs[2 * b + 1]
        v4 = o0["out_v"] + o1["out_v"]
        v = v4.reshape(4, 32, HA)[:, :K, :].sum(axis=0)
        mnum = o0["out_mnum"] + o1["out_mnum"]
        memb = o0["out_memb"]
        length = lengths[b]

        link_rep = v[:, :H] / (NH * length[:, None])
        m_ctx = mnum[:, :H] / (mnum[:, H:H + 1] + NH * 1e-5)
        enum = mnum.reshape(E, MPE, HA).sum(axis=1)
        e_ctx = enum[:, :H] / (enum[:, H:H + 1] + NH * MPE * 1e-5)

        mg = memb.reshape(E, MPE, H)
        mmax = mg.max(axis=1)
        eemb = np.log(np.exp(mg - mmax[:, None, :]).sum(axis=1)) + mmax

        nodes_raw = np.concatenate([eemb, memb, link_rep], axis=0)      # [176,H]
        nodes = np.concatenate([nodes_raw, nodes_type], axis=1)         # [176,H+20]
        ctx = np.concatenate([e_ctx, m_ctx], axis=0)                    # [160,H]
        ctx = np.concatenate([ctx, np.zeros((E + EM, TYPE_DIM), np.float32)], axis=1)
        out[b] = np.concatenate([nodes, ctx], axis=0)
    return out


def kernel(**inputs):
    from concourse.bass_utils import run_bass_kernel_spmd

    in_maps, lengths, nchunk = _per_core_inputs(
        inputs["sequence_output"], inputs["attention"],
        inputs["mention_pos"], inputs["link_start"], inputs["link_len"])
    nc = _get_nc(nchunk)
    res = run_bass_kernel_spmd(nc, in_maps, core_ids=list(range(8)))
    return _combine(res.results, lengths, inputs["type_table"])


# revision 22
# speedup vs baseline: 1.9912x; 1.1150x over previous
"""Trainium2 Bass kernel for nn_DocREModel (doc-level relation extraction graph pooling).

Strategy (8 NeuronCores): each doc b (B=4) is split across 2 cores by attention
heads (6 heads each).  Key observation: the model only ever reads attention rows
at mention positions (<=128 distinct) and inside link spans, i.e. ~35% of the
[1024,1024] matrix.  Each core therefore device-GATHERS just those rows
(SWDGE indirect DMA with a runtime per-partition index tile), head-sums them on
the vector engine, and runs small PE matmuls:

  - att_r HBM layout [row, head*1024+col] (bf16): gathering one row pulls all 6
    heads of that row contiguously (12 KB/descriptor).
  - gather slot j<128 == mention j (EM=128 exactly), so the "onehot" gather
    matrix is the identity: mention rows of S come from a PE transpose of
    gathered chunk 0.  Span rows occupy slots >=128; a host-built slot mask
    [slots,16] reduces them to per-span row-sums uT[16,L] via PE matmul
    (transposed so each PSUM bank holds a single accumulation group).
  - mnum = S_mention^T @ [seq|1]  (context numerators + row-sums)
  - v    = (uT*maskCT)^T-transposed-back @ [seq|1], col-group-tiled 4x on the
    PE (16-row outputs packed at partition offsets 0/32/64/96; host sums).
  - memb = seq rows at mentions via a second (f32) indirect gather - no compute.
The host applies the tiny normalizations (head-count / span-length / row-sum
divides, entity pooling, 4-way logsumexp) while unsharding.
"""

import os
import sys

for _p in ("/opt/trn_rl_repo", "/root/.axon_site/_ro/trn_rl_repo"):
    if os.path.isdir(_p) and _p not in sys.path:
        sys.path.insert(0, _p)

import numpy as np

B, L, H, NH = 4, 1024, 768, 12
E, MPE, K = 32, 4, 16
EM = E * MPE              # 128 mentions per doc == gather chunk 0
TYPE_DIM = 20
OFFSET = 1
HPG = NH // 2             # heads per core (2 cores per doc)
CT = L // 128             # 8 column chunks
HA = H + 4                # seq | ones | 3 zero-pad -> 772 (row-sum in col 768)
NCHUNK_DEF = 3            # gather slots = 128*NCHUNK (>= 128 mentions + span rows)


def _build_nc(nchunk=NCHUNK_DEF, debug=False):
    import concourse.bass as bass
    import concourse.mybir as mybir
    import concourse.tile as tile
    from concourse import bacc

    f32 = mybir.dt.float32
    bf16 = mybir.dt.bfloat16
    fp8 = mybir.dt.float8e4  # e4m3
    i32 = mybir.dt.int32
    ts, ds = bass.ts, bass.ds

    nc = bacc.Bacc("TRN2", target_bir_lowering=False, debug=debug)

    att_r = nc.dram_tensor("att_r", [L, HPG * L], fp8, kind="ExternalInput")
    seqb = nc.dram_tensor("seqb", [L, HA], bf16, kind="ExternalInput")
    seqf = nc.dram_tensor("seqf", [L, H], f32, kind="ExternalInput")
    idx_att = nc.dram_tensor("idx_att", [128, nchunk], i32, kind="ExternalInput")
    idx_mem = nc.dram_tensor("idx_mem", [128, 1], i32, kind="ExternalInput")
    maskS = nc.dram_tensor("maskS", [128, nchunk * K], bf16, kind="ExternalInput")
    maskCT = nc.dram_tensor("maskCT", [K, L], f32, kind="ExternalInput")
    ident = nc.dram_tensor("ident", [128, 128], bf16, kind="ExternalInput")
    ident8 = nc.dram_tensor("ident8", [128, 128], fp8, kind="ExternalInput")
    out_mnum = nc.dram_tensor("out_mnum", [EM, HA], bf16, kind="ExternalOutput")
    out_v = nc.dram_tensor("out_v", [4 * K, HA], bf16, kind="ExternalOutput")
    out_memb = nc.dram_tensor("out_memb", [EM, H], f32, kind="ExternalOutput")

    with tile.TileContext(nc) as tc:
        with (
            tc.tile_pool(name="const", bufs=1) as constp,
            tc.tile_pool(name="gat", bufs=3) as gatp,
            tc.tile_pool(name="acc", bufs=1) as accp,
            tc.tile_pool(name="stage", bufs=1) as stagep,
            tc.tile_pool(name="pshold", bufs=1, space="PSUM") as pshold,
            tc.tile_pool(name="psrot", bufs=2, space="PSUM") as psrot,
        ):
            # ---- indices first: gathers are the critical path ----
            idxa_s = constp.tile([128, nchunk], i32, tag="idxa", name="idxa")
            idxm_s = constp.tile([128, 1], i32, tag="idxm", name="idxm")
            nc.sync.dma_start(out=idxa_s[:], in_=idx_att[:])
            nc.sync.dma_start(out=idxm_s[:], in_=idx_mem[:])

            g5s = []
            for q in range(nchunk):
                g5 = gatp.tile([128, HPG * L], fp8, tag="g5", name="g5")
                nc.gpsimd.indirect_dma_start(
                    out=g5[:], out_offset=None, in_=att_r[:],
                    in_offset=bass.IndirectOffsetOnAxis(ap=idxa_s[:, q:q + 1], axis=0))
                g5s.append(g5)

            # ---- mention embeddings: pure f32 row-gather of seq ----
            memb_s = stagep.tile([128, H], f32, tag="memb", name="memb")
            nc.gpsimd.indirect_dma_start(
                out=memb_s[:], out_offset=None, in_=seqf[:],
                in_offset=bass.IndirectOffsetOnAxis(ap=idxm_s[:], axis=0))
            nc.sync.dma_start(out=out_memb[:], in_=memb_s[:])

            # ---- remaining consts ----
            maskS_s = constp.tile([128, nchunk, K], bf16, tag="maskS", name="maskS")
            maskCT_s = constp.tile([K, L], f32, tag="maskCT", name="maskCT")
            ident_s = constp.tile([128, 128], bf16, tag="ident", name="ident")
            ident8_s = constp.tile([128, 128], fp8, tag="ident8", name="ident8")
            seq_s = constp.tile([128, CT, HA], bf16, tag="seqs", name="seqs")
            nc.sync.dma_start(out=maskS_s[:], in_=maskS[:].rearrange("p (q k) -> p q k", k=K))
            nc.scalar.dma_start(out=maskCT_s[:], in_=maskCT[:])
            nc.sync.dma_start(out=ident_s[:], in_=ident[:])
            nc.sync.dma_start(out=ident8_s[:], in_=ident8[:])
            nc.scalar.dma_start(out=seq_s[:], in_=seqb[:].rearrange("(c p) f -> p c f", p=128))

            # ---- head-sum on PE (identity-weight accumulation over 6 heads) ----
            gs_s = accp.tile([128, nchunk, L], bf16, tag="gs", name="gs")
            gtm_s = accp.tile([128, CT, 128], bf16, tag="gtm", name="gtm")
            wv_s = accp.tile([128, CT, K], bf16, tag="wv", name="wv")
            put0 = pshold.tile([K, 512], f32, tag="put0", name="put0")
            put1 = pshold.tile([K, 512], f32, tag="put1", name="put1")
            for q in range(nchunk):
                g5 = g5s[q]
                for half in range(2):
                    ph = psrot.tile([128, 512], f32, tag="ph", name="ph")
                    for h in range(HPG):
                        nc.tensor.matmul(ph[:], ident8_s[:],
                                         g5[:, ds(h * L + half * 512, 512)],
                                         start=(h == 0), stop=(h == HPG - 1))
                    nc.scalar.copy(out=gs_s[:, q, ds(half * 512, 512)], in_=ph[:])
                # span-row reduction (transposed): uT[k,c] += maskS^T @ gs_q
                nc.tensor.matmul(put0[:], maskS_s[:, q, :], gs_s[:, q, 0:512],
                                 start=(q == 0), stop=(q == nchunk - 1))
                nc.tensor.matmul(put1[:], maskS_s[:, q, :], gs_s[:, q, 512:L],
                                 start=(q == 0), stop=(q == nchunk - 1))
                if q == 0:
                    # mention rows of S: PE-transpose of gathered chunk 0
                    for ct in range(CT):
                        pt = psrot.tile([128, 128], bf16, tag="pt", name="pt")
                        nc.tensor.transpose(pt[:], gs_s[:, 0, ts(ct, 128)], ident_s[:])
                        nc.vector.tensor_copy(gtm_s[:, ct, :], pt[:])

            # ---- mnum: mention-context numerators (runs early, off gtm) ----
            pm0 = pshold.tile([EM, 512], f32, tag="pm0", name="pm0")
            pm1 = pshold.tile([EM, HA - 512], f32, tag="pm1", name="pm1")
            for ct in range(CT):
                st, sp = (ct == 0), (ct == CT - 1)
                nc.tensor.matmul(pm0[:], gtm_s[:, ct, :], seq_s[:, ct, 0:512], start=st, stop=sp)
                nc.tensor.matmul(pm1[:], gtm_s[:, ct, :], seq_s[:, ct, 512:HA], start=st, stop=sp)
            mnum_s = stagep.tile([EM, HA], bf16, tag="mnum", name="mnum")
            nc.scalar.copy(out=mnum_s[:, 0:512], in_=pm0[:])
            nc.scalar.copy(out=mnum_s[:, 512:HA], in_=pm1[:])
            nc.sync.dma_start(out=out_mnum[:], in_=mnum_s[:])

            # ---- wvT = uT * column-mask, transpose back, col-tiled v matmuls ----
            wvt_s = accp.tile([K, L], bf16, tag="wvt", name="wvt")
            nc.vector.tensor_mul(wvt_s[:, 0:512], put0[:], maskCT_s[:, 0:512])
            nc.vector.tensor_mul(wvt_s[:, 512:L], put1[:], maskCT_s[:, 512:L])
            for ct in range(CT):
                ptk = psrot.tile([128, 128], bf16, tag="pt", name="ptk")
                nc.tensor.transpose(ptk[:, 0:K], wvt_s[:, ts(ct, 128)], ident_s[0:K, 0:K])
                nc.vector.tensor_copy(wv_s[:, ct, :], ptk[:, 0:K])
            pv0 = psrot.tile([128, 512], f32, tag="ph", name="pv0")
            pv1 = psrot.tile([128, 512], f32, tag="ph", name="pv1")
            for ct in range(CT):
                grp = ct // 2
                st, sp = (ct % 2 == 0), (ct % 2 == 1)
                nc.tensor.matmul(pv0[ds(32 * grp, K), :], wv_s[:, ct, :],
                                 seq_s[:, ct, 0:512], start=st, stop=sp,
                                 tile_position=(0, 32 * grp))
                nc.tensor.matmul(pv1[ds(32 * grp, K), 0:HA - 512], wv_s[:, ct, :],
                                 seq_s[:, ct, 512:HA], start=st, stop=sp,
                                 tile_position=(0, 32 * grp))
            v_s = stagep.tile([128, HA], bf16, tag="v", name="v")
            for grp in range(4):
                nc.scalar.copy(out=v_s[ds(32 * grp, K), 0:512], in_=pv0[ds(32 * grp, K), :])
                nc.vector.tensor_copy(v_s[ds(32 * grp, K), 512:HA],
                                      pv1[ds(32 * grp, K), 0:HA - 512])
            for grp in range(4):
                nc.scalar.dma_start(out=out_v[ts(grp, K), :], in_=v_s[ds(32 * grp, K), :])

    nc.compile()
    return nc


_NC_CACHE = {}


def _get_nc(nchunk=NCHUNK_DEF):
    if nchunk not in _NC_CACHE:
        _NC_CACHE[nchunk] = _build_nc(nchunk)
    return _NC_CACHE[nchunk]


def _per_core_inputs(sequence_output, attention, mention_pos, link_start, link_len):
    """Returns (in_maps for 8 cores, per-doc span lengths, nchunk)."""
    import ml_dtypes
    seq = np.ascontiguousarray(np.asarray(sequence_output, dtype=np.float32))
    att = np.asarray(attention)
    mpos = np.asarray(mention_pos).astype(np.int64)
    lstart = np.asarray(link_start).astype(np.int64)
    llen = np.asarray(link_len).astype(np.int64)

    docs = []
    max_slots = 0
    for b in range(B):
        pos = (mpos[b] + OFFSET).reshape(EM)
        s = lstart[b] + OFFSET
        e = lstart[b] + llen[b] + 1 + OFFSET
        row2slot = {}
        slots = list(pos)
        for j, r in enumerate(pos):
            row2slot.setdefault(int(r), j)
        for si, ei in zip(s, e):
            for r in range(int(si), int(ei)):
                if r not in row2slot:
                    row2slot[r] = len(slots)
                    slots.append(r)
        docs.append((pos, s, e, row2slot, slots))
        max_slots = max(max_slots, len(slots))
    nchunk = max(NCHUNK_DEF, -(-max_slots // 128))

    ident = np.eye(128, dtype=ml_dtypes.bfloat16)
    ident8 = np.eye(128, dtype=ml_dtypes.float8_e4m3fn)
    in_maps = []
    lengths = []
    for b in range(B):
        pos, s, e, row2slot, slots = docs[b]
        n_slots = nchunk * 128
        sl = np.zeros(n_slots, np.int32)
        sl[:len(slots)] = slots
        idx_att = np.ascontiguousarray(sl.reshape(nchunk, 128).T)
        idx_mem = np.ascontiguousarray(pos.astype(np.int32).reshape(128, 1))
        mS = np.zeros((n_slots, K), np.float32)
        mC = np.zeros((L, K), np.float32)
        for k, (si, ei) in enumerate(zip(s, e)):
            mC[int(si):int(ei), k] = 1.0
            for r in range(int(si), int(ei)):
                mS[row2slot[r], k] = 1.0
        maskS = np.ascontiguousarray(
            mS.reshape(nchunk, 128, K).transpose(1, 0, 2).reshape(128, nchunk * K)
        ).astype(ml_dtypes.bfloat16)
        maskCT = np.ascontiguousarray(mC.T)
        seqb = np.concatenate(
            [seq[b], np.ones((L, 1), np.float32), np.zeros((L, HA - H - 1), np.float32)],
            axis=1).astype(ml_dtypes.bfloat16)
        seqf = seq[b]
        lengths.append((e - s).astype(np.float32))
        for g in range(2):
            att_r = np.ascontiguousarray(
                att[b, g * HPG:(g + 1) * HPG].astype(ml_dtypes.float8_e4m3fn)
                .transpose(1, 0, 2).reshape(L, HPG * L))
            in_maps.append({
                "att_r": att_r, "seqb": seqb, "seqf": seqf,
                "idx_att": idx_att, "idx_mem": idx_mem,
                "maskS": maskS, "maskCT": maskCT, "ident": ident, "ident8": ident8,
            })
    return in_maps, lengths, nchunk


def _combine(outs, lengths, type_table):
    ttab = np.asarray(type_table, dtype=np.float32)
    type_ids = np.concatenate(
        [np.zeros(E, np.int64), np.ones(EM, np.int64), np.full(K, 2, np.int64)])
    nodes_type = ttab[type_ids]  # [E+EM+K, TYPE_DIM]

    out = np.zeros((B, E + EM + K + E + EM, H + TYPE_DIM), np.float32)
    for b in range(B):
        o0, o1 = outs[2 * b], outs[2 * b + 1]
        v4 = np.asarray(o0["out_v"], np.float32) + np.asarray(o1["out_v"], np.float32)
        v = v4.reshape(4, K, HA).sum(axis=0)
        mnum = (np.asarray(o0["out_mnum"], np.float32)
                + np.asarray(o1["out_mnum"], np.float32))
        memb = o0["out_memb"]
        length = lengths[b]

        link_rep = v[:, :H] / (NH * length[:, None])
        m_ctx = mnum[:, :H] / (mnum[:, H:H + 1] + NH * 1e-5)
        enum = mnum.reshape(E, MPE, HA).sum(axis=1)
        e_ctx = enum[:, :H] / (enum[:, H:H + 1] + NH * MPE * 1e-5)

        mg = memb.reshape(E, MPE, H)
        mmax = mg.max(axis=1)
        eemb = np.log(np.exp(mg - mmax[:, None, :]).sum(axis=1)) + mmax

        nodes_raw = np.concatenate([eemb, memb, link_rep], axis=0)      # [176,H]
        nodes = np.concatenate([nodes_raw, nodes_type], axis=1)         # [176,H+20]
        ctx = np.concatenate([e_ctx, m_ctx], axis=0)                    # [160,H]
        ctx = np.concatenate([ctx, np.zeros((E + EM, TYPE_DIM), np.float32)], axis=1)
        out[b] = np.concatenate([nodes, ctx], axis=0)
    return out


def kernel(**inputs):
    from concourse.bass_utils import run_bass_kernel_spmd

    in_maps, lengths, nchunk = _per_core_inputs(
        inputs["sequence_output"], inputs["attention"],
        inputs["mention_pos"], inputs["link_start"], inputs["link_len"])
    nc = _get_nc(nchunk)
    res = run_bass_kernel_spmd(nc, in_maps, core_ids=list(range(8)))
    return _combine(res.results, lengths, inputs["type_table"])
